# revision 33
# baseline (speedup 1.0000x reference)
"""Committee-of-linear-classifiers vote histogram on 8 Trainium2 cores.

Computation (per sample b):
    logits[m, c] = x[b] . W[m, :, c] + b[m, c]      (16 models, 10 classes)
    vote[m] = argmax_c logits[m, c]
    hist[b, c] = #{m : vote[m] == c}

Final design (data-parallel, 8192 samples/core, W/b replicated).
Measured ~60us HW exec (baseline was 93us). Key mechanisms:

  - Host prep: fp16 cast + partition-major relayout so every DMA is
    per-partition contiguous (128 descriptors x multi-KiB).
  - DMA: each HWDGE ring (sync/scalar) pays a ~2.8us completion receipt
    per transfer regardless of size, so x streams as just 5 chunks
    [1024, 1024, 2048, 2048, 2048] samples alternating scalar/sync,
    sized so each lands before the compute stream reaches it. W and the
    bias ride in ONE dram tensor (a separate tiny bias DMA would pay a
    full receipt and gate the bias matmuls). GpSimd's SWDGE ring is NOT
    used: its Q7 cores are busy with tree adds.
  - PE: per 128-sample tile j: 4 K-chunk matmuls (fp16, fp32 PSUM
    accumulate), (k, j)-interleaved so accumulation drains overlap the
    next fill; bias via ONE K=1 ones-matmul per PSUM bank (two j-tiles
    share a bank, bias repeated twice -- LDWEIGHTS costs ~columns
    regardless of K, so per-tile K=1 matmuls are as costly as full
    pairs). A warmup matmul burst on W trips the PE HAM clock gate
    (1.2 -> 2.4 GHz) before real work; with a backlogged tensor queue
    the LDWEIGHTS of the next matmul hides under the current one
    (~70-100ns per 160-col pair).
  - PSUM: unit = 4 sample tiles (512 samples) in one 2-bank tile
    (j offsets 0/160/512/672), bufs=4 = all 8 banks; ONE strided
    ScalarE copy stages the whole unit to SBUF fp16.
  - Postprocess, balanced across three engines at ~1.7us/unit each:
      ACT   copy PSUM->SBUF fp16; broadcast-materialize the row max
            (stride-0 operands would force the DVE compare into 1x mode)
      DVE   reduce_max (1x, unavoidable); is_ge vs materialized max
            (2x mode, ~485ns); last two 2->1 tree adds
      GpSimd 16->4 model pairwise adds (own SBUF port slot, runs
            concurrently with DVE 1x/2x_1P ops)
    Emission is software-pipelined two units deep: unit u's broadcast/
    compare/tree is emitted after unit u+2's matmuls, so the strict
    per-engine FIFOs always hold ready work and no cross-engine
    round-trip latency shows in the cadence (~2.0us/unit).
  - Output: per-4-unit group DMA of the fp16 histograms; host undoes
    the layout and upcasts (counts <= 16 are exact in fp16).
"""

import os
import sys

import numpy as np

if "/opt/trn_rl_repo" not in sys.path:
    sys.path.insert(0, "/opt/trn_rl_repo")

NCORES = 8
B, D, M, C = 65536, 512, 16, 10
MC = M * C            # 160
BL = B // NCORES      # 8192 samples per core
KCH = D // 128        # 4 contraction chunks
CHUNKS = [1024, 1024, 2048, 2048, 2048]  # chunk sizes (samples)
NU = 16               # postprocess units per core
US = BL // NU         # 512 samples per unit
NJU = US // 128       # 4 sample tiles per unit
WARMUP_MM = 14       # PE HAM warmup matmuls

assert sum(CHUNKS) == BL

_NC_CACHE = {}
LAST_RESULT = None  # BassKernelResults of the most recent run (for test harness)


def build_nc():
    key = "v3"
    if key in _NC_CACHE:
        return _NC_CACHE[key]

    from contextlib import ExitStack

    import concourse.bacc as bacc
    import concourse.tile as tile
    from concourse import mybir

    fp16 = mybir.dt.float16
    fp32 = mybir.dt.float32
    bf16 = mybir.dt.bfloat16

    nc = bacc.Bacc("TRN2", target_bir_lowering=False, debug=False,
                   enable_asserts=False)
    xh = nc.dram_tensor("xh", [128, KCH * BL], fp16, kind="ExternalInput").ap()
    # W and bias ride in ONE dram tensor / one DMA: a separate tiny bias DMA
    # pays the full ~2-3us ring completion receipt and gates the bias matmuls
    wb = nc.dram_tensor("wb", [128, (KCH + 2) * MC], fp16,
                        kind="ExternalInput").ap()
    out = nc.dram_tensor("out", [128, NU * NJU * C], fp16,
                         kind="ExternalOutput").ap()

    # sample offset of each chunk
    choff = [0]
    for c in CHUNKS:
        choff.append(choff[-1] + c)
    # which chunk supplies each unit, and the in-chunk sample offset
    unit_src = []
    for u in range(NU):
        s = u * US
        ci = max(i for i in range(len(CHUNKS)) if choff[i] <= s)
        unit_src.append((ci, s - choff[ci]))

    with tile.TileContext(nc) as tc, ExitStack() as ctx:
        wpool = ctx.enter_context(tc.tile_pool(name="wpool", bufs=1))
        xpool = ctx.enter_context(tc.tile_pool(name="xpool", bufs=1))
        ppool = ctx.enter_context(tc.tile_pool(name="ppool", bufs=4,
                                               space="PSUM"))
        tpool = ctx.enter_context(tc.tile_pool(name="tpool", bufs=6))
        gpool = ctx.enter_context(tc.tile_pool(name="gpool", bufs=6))
        mpool = ctx.enter_context(tc.tile_pool(name="mpool", bufs=6))
        spool = ctx.enter_context(tc.tile_pool(name="spool", bufs=6))
        opool = ctx.enter_context(tc.tile_pool(name="opool", bufs=1))

        # small inputs first (they gate warmup + everything else)
        whsb = wpool.tile([128, (KCH + 2) * MC], fp16)
        nc.sync.dma_start(whsb, wb)
        whs = whsb[:, 0:KCH * MC].rearrange("p (k n) -> p k n", k=KCH)
        bst2 = whsb[0:1, KCH * MC:(KCH + 2) * MC]  # bias pattern, twice
        ones1 = wpool.tile([1, 128], fp16)
        nc.gpsimd.memset(ones1, 1.0)

        # x: one persistent SBUF tile, streamed in contiguous chunks.
        # layout per chunk: [128, KCH, chs] at flat offset 4*choff[c]
        xt = xpool.tile([128, KCH * BL], fp16)
        # 3 DMA rings in parallel: each ring pays a ~2us completion receipt
        # per transfer (size-independent), so one ring moves only ~1MB/4.5us.
        # c0 leads the scalar ring (nothing ahead of it -> earliest landing);
        # gpsimd's issues all happen before its first tree op (~13us), and
        # each chunk lands before the unit stream reaches it.
        ring = [nc.scalar, nc.sync, nc.scalar, nc.sync, nc.scalar]
        for ci, chs in enumerate(CHUNKS):
            o = KCH * choff[ci]
            n = KCH * chs
            ring[ci].dma_start(xt[:, o:o + n], xh[:, o:o + n])

        def x_tile(ci, s_in_chunk, k):
            # lhsT [128, 128] for k-chunk k, samples s..s+127 of chunk ci
            o = KCH * choff[ci] + k * CHUNKS[ci] + s_in_chunk
            return xt[:, o:o + 128]

        # PE HAM warmup: junk matmuls on W data into a to-be-recycled PSUM
        # bank; keeps the clock at 2.4 GHz by the time real tiles arrive.
        ps = ppool.tile([128, 2 * 512], fp32)  # same tag as the unit tiles
        for i in range(WARMUP_MM):
            nc.tensor.matmul(ps[:, 0:MC], lhsT=whs[:, 0, 0:128],
                             rhs=whs[:, i % KCH, :], start=True, stop=True)

        outbuf = opool.tile([128, NU, NJU, C], fp16)
        JOFF = (0, MC, 512, 512 + MC)   # j-tile offsets inside the 2-bank ps
        stash = {}   # per-unit (t, t3, mx) awaiting the compare stage
        stash2 = {}  # per-unit p2 awaiting the tree tail (one unit later)

        def emit_front(u):
            """Matmuls + PSUM staging + max + broadcast for unit u."""
            ci, s0 = unit_src[u]
            ps = ppool.tile([128, 2 * 512], fp32)
            # one bias matmul per bank covers two j-tiles (N=320)
            for bank in range(2):
                nc.tensor.matmul(ps[:, bank * 512:bank * 512 + 2 * MC],
                                 lhsT=ones1, rhs=bst2,
                                 start=True, stop=False)
            # (k, j) interleave: consecutive matmuls hit different PSUM
            # regions, so one accumulation's drain overlaps the next's fill
            for k in range(KCH):
                for jj in range(NJU):
                    po = ps[:, JOFF[jj]:JOFF[jj] + MC]
                    nc.tensor.matmul(
                        po, lhsT=x_tile(ci, s0 + jj * 128, k),
                        rhs=whs[:, k, :], start=False, stop=(k == KCH - 1))

            # stage the whole unit PSUM -> SBUF fp16 in one strided copy
            t = tpool.tile([128, NJU, M, C], fp16)
            ps_v = (ps.rearrange("p (b x) -> p b x", b=2)[:, :, 0:2 * MC]
                    .rearrange("p b (j n) -> p b j n", j=2))
            nc.scalar.copy(t.rearrange("p (b j) m c -> p b j (m c)", b=2),
                           ps_v)

            t3 = t.rearrange("p j m c -> p (j m) c")
            mx = mpool.tile([128, NJU * M], fp16)
            nc.vector.reduce_max(mx, t3, axis=mybir.AxisListType.X)
            stash[u] = (t, t3, mx)

        def emit_back(u, drain=False):
            """Broadcast + compare + vote-histogram tree for unit u.
            Emitted one unit late so the ACT queue's copy(u+1) is never
            stuck behind bcast(u) (which waits on DVE's max)."""
            t, t3, mx = stash.pop(u)
            mxr = mpool.tile([128, NJU * M, C], fp16)
            nc.scalar.copy(mxr,
                           mx.unsqueeze(2).broadcast_to((128, NJU * M, C)))
            ge = gpool.tile([128, NJU, M, C], bf16)
            nc.vector.tensor_tensor(
                ge.rearrange("p j m c -> p (j m) c"), t3,
                mxr.rearrange("p x c -> p (x c)"),
                mybir.AluOpType.is_ge)
            # histogram tree: GpSimd in steady state (overlaps DVE work on
            # other units); DVE-only in drain mode (contention-free 2x adds
            # beat the cross-engine pipeline when nothing else runs)
            eng = nc.vector if drain else nc.gpsimd
            with nc.allow_low_precision("histogram counts are small ints"):
                p1 = spool.tile([128, NJU, 8, C], bf16)
                eng.tensor_tensor(p1, ge[:, :, 0:8, :], ge[:, :, 8:16, :],
                                  mybir.AluOpType.add)
                p2 = spool.tile([128, NJU, 4, C], bf16)
                eng.tensor_tensor(p2, p1[:, :, 0:4, :], p1[:, :, 4:8, :],
                                  mybir.AluOpType.add)
            stash2[u] = p2

        def emit_tail(u):
            """Final 4->1 adds on DVE, one unit behind the GpSimd levels so
            the DVE queue never waits on the GpSimd round-trip."""
            p2 = stash2.pop(u)
            with nc.allow_low_precision("histogram counts are small ints"):
                nc.vector.reduce_sum(
                    outbuf[:, u], p2.rearrange("p j m c -> p j c m"),
                    axis=mybir.AxisListType.X)
            if u % 4 == 3:  # stream the finished group out
                g = u // 4
                nc.sync.dma_start(
                    out[:, g * 4 * NJU * C:(g + 1) * 4 * NJU * C],
                    outbuf[:, g * 4:(g + 1) * 4].rearrange(
                        "p u j c -> p (u j c)"))

        # two-unit software pipeline: unit u's broadcast/compare stage is
        # emitted after unit u+2's front, so every queue always has ready
        # work while a unit's cross-engine round-trips are in flight
        for u in range(NU):
            emit_front(u)
            if u > 2:
                emit_back(u - 3, drain=(u - 3 >= NU - 6))
            if u > 3:
                emit_tail(u - 4)
        for u in range(NU - 3, NU):
            emit_back(u, drain=True)
            emit_tail(u - 1)
        emit_tail(NU - 1)

    nc.compile()
    _NC_CACHE[key] = nc
    return nc


def make_in_maps(x, W, b, ncores=NCORES):
    """Host-side prep: fp16 cast + partition-major relayout + sharding."""
    x16 = np.asarray(x, dtype=np.float32).astype(np.float16)        # [B, D]
    w16 = np.ascontiguousarray(
        np.asarray(W, dtype=np.float32).transpose(1, 0, 2).reshape(D, MC)
    ).astype(np.float16)                                            # [D, 160]
    b16 = np.asarray(b, dtype=np.float32).reshape(MC).astype(np.float16)

    # w dev layout: [p, k, mc] with d = k*128 + p
    w_dev = (w16.reshape(KCH, 128, MC).transpose(1, 0, 2)
             .reshape(128, KCH * MC))
    b_rep = np.broadcast_to(np.concatenate([b16, b16]), (128, 2 * MC))
    wb_dev = np.ascontiguousarray(
        np.concatenate([w_dev, b_rep], axis=1))

    in_maps = []
    for ci in range(ncores):
        xc = x16[ci * BL:(ci + 1) * BL]                             # [BL, D]
        # per chunk: [128, KCH, chs] with dev[p, k, s] = x[b0+s, k*128+p]
        slabs = []
        o = 0
        for chs in CHUNKS:
            slab = xc[o:o + chs].reshape(chs, KCH, 128).transpose(2, 1, 0)
            slabs.append(np.ascontiguousarray(slab).reshape(128, KCH * chs))
            o += chs
        xd = np.ascontiguousarray(np.concatenate(slabs, axis=1))
        in_maps.append({"xh": xd, "wb": wb_dev})
    return in_maps


def kernel(x, W, b):
    global LAST_RESULT
    from concourse import bass_utils

    # NTFF tracing under axon needs the antenv.axon_hooks shim; without it
    # run_bass_kernel_spmd(trace=True) raises. Disable tracing defensively
    # when the hook module is absent (BASS_TRACE may be set in the env).
    want_trace = bool(os.environ.get("BASS_TRACE"))
    try:
        from antenv.axon_hooks import get_axon_ntff_profile_hook  # noqa: F401
    except ImportError:
        want_trace = False
        os.environ["BASS_NEVER_TRACE"] = "1"

    in_maps = make_in_maps(x, W, b)
    nc = build_nc()
    res = bass_utils.run_bass_kernel_spmd(
        nc, in_maps, core_ids=list(range(NCORES)),
        trace=want_trace,
    )
    LAST_RESULT = res
    # device out [128, NU, NJU, C] -> rows b = u*US + j*128 + p
    outs = []
    for r in res.results:
        o = r["out"].reshape(128, NU, NJU, C)
        outs.append(o.transpose(1, 2, 0, 3).reshape(BL, C))
    return np.concatenate(outs, axis=0).astype(np.float32)


# revision 34
# speedup vs baseline: 1.0005x; 1.0005x over previous
"""Committee-of-linear-classifiers vote histogram on 8 Trainium2 cores.

Computation (per sample b):
    logits[m, c] = x[b] . W[m, :, c] + b[m, c]      (16 models, 10 classes)
    vote[m] = argmax_c logits[m, c]
    hist[b, c] = #{m : vote[m] == c}

Final design (data-parallel, 8192 samples/core, W/b replicated).
Measured ~60us HW exec (baseline was 93us). Key mechanisms:

  - Host prep: fp16 cast + partition-major relayout so every DMA is
    per-partition contiguous (128 descriptors x multi-KiB).
  - DMA: each HWDGE ring (sync/scalar) pays a ~2.8us completion receipt
    per transfer regardless of size, so x streams as just 5 chunks
    [1024, 1024, 2048, 2048, 2048] samples alternating scalar/sync,
    sized so each lands before the compute stream reaches it. W and the
    bias ride in ONE dram tensor (a separate tiny bias DMA would pay a
    full receipt and gate the bias matmuls). GpSimd's SWDGE ring is NOT
    used: its Q7 cores are busy with tree adds.
  - PE: per 128-sample tile j: 4 K-chunk matmuls (fp16, fp32 PSUM
    accumulate), (k, j)-interleaved so accumulation drains overlap the
    next fill; bias via ONE K=1 ones-matmul per PSUM bank (two j-tiles
    share a bank, bias repeated twice -- LDWEIGHTS costs ~columns
    regardless of K, so per-tile K=1 matmuls are as costly as full
    pairs). A warmup matmul burst on W trips the PE HAM clock gate
    (1.2 -> 2.4 GHz) before real work; with a backlogged tensor queue
    the LDWEIGHTS of the next matmul hides under the current one
    (~70-100ns per 160-col pair).
  - PSUM: unit = 4 sample tiles (512 samples) in one 2-bank tile
    (j offsets 0/160/512/672), bufs=4 = all 8 banks; ONE strided
    ScalarE copy stages the whole unit to SBUF fp16.
  - Postprocess, balanced across three engines at ~1.7us/unit each:
      ACT   copy PSUM->SBUF fp16; broadcast-materialize the row max
            (stride-0 operands would force the DVE compare into 1x mode)
      DVE   reduce_max (1x, unavoidable); is_ge vs materialized max
            (2x mode, ~485ns); last two 2->1 tree adds
      GpSimd 16->4 model pairwise adds (own SBUF port slot, runs
            concurrently with DVE 1x/2x_1P ops)
    Emission is software-pipelined two units deep: unit u's broadcast/
    compare/tree is emitted after unit u+2's matmuls, so the strict
    per-engine FIFOs always hold ready work and no cross-engine
    round-trip latency shows in the cadence (~2.0us/unit).
  - Output: per-4-unit group DMA of the fp16 histograms; host undoes
    the layout and upcasts (counts <= 16 are exact in fp16).
"""

import os
import sys

import numpy as np

if "/opt/trn_rl_repo" not in sys.path:
    sys.path.insert(0, "/opt/trn_rl_repo")

NCORES = 8
B, D, M, C = 65536, 512, 16, 10
MC = M * C            # 160
BL = B // NCORES      # 8192 samples per core
KCH = D // 128        # 4 contraction chunks
CHUNKS = [1024, 1024, 2048, 2048, 2048]  # chunk sizes (samples)
NU = 16               # postprocess units per core
US = BL // NU         # 512 samples per unit
NJU = US // 128       # 4 sample tiles per unit
WARMUP_MM = 14       # PE HAM warmup matmuls

assert sum(CHUNKS) == BL

_NC_CACHE = {}
LAST_RESULT = None  # BassKernelResults of the most recent run (for test harness)


def build_nc():
    key = "v3"
    if key in _NC_CACHE:
        return _NC_CACHE[key]

    from contextlib import ExitStack

    import concourse.bacc as bacc
    import concourse.tile as tile
    from concourse import mybir

    fp16 = mybir.dt.float16
    fp32 = mybir.dt.float32
    bf16 = mybir.dt.bfloat16

    nc = bacc.Bacc("TRN2", target_bir_lowering=False, debug=False,
                   enable_asserts=False)
    xh = nc.dram_tensor("xh", [128, KCH * BL], fp16, kind="ExternalInput").ap()
    # W and bias ride in ONE dram tensor / one DMA: a separate tiny bias DMA
    # pays the full ~2-3us ring completion receipt and gates the bias matmuls
    wb = nc.dram_tensor("wb", [128, (KCH + 2) * MC], fp16,
                        kind="ExternalInput").ap()
    out = nc.dram_tensor("out", [128, NU * NJU * C], fp16,
                         kind="ExternalOutput").ap()

    # sample offset of each chunk
    choff = [0]
    for c in CHUNKS:
        choff.append(choff[-1] + c)
    # which chunk supplies each unit, and the in-chunk sample offset
    unit_src = []
    for u in range(NU):
        s = u * US
        ci = max(i for i in range(len(CHUNKS)) if choff[i] <= s)
        unit_src.append((ci, s - choff[ci]))

    with tile.TileContext(nc) as tc, ExitStack() as ctx:
        wpool = ctx.enter_context(tc.tile_pool(name="wpool", bufs=1))
        xpool = ctx.enter_context(tc.tile_pool(name="xpool", bufs=1))
        ppool = ctx.enter_context(tc.tile_pool(name="ppool", bufs=4,
                                               space="PSUM"))
        tpool = ctx.enter_context(tc.tile_pool(name="tpool", bufs=6))
        gpool = ctx.enter_context(tc.tile_pool(name="gpool", bufs=6))
        mpool = ctx.enter_context(tc.tile_pool(name="mpool", bufs=6))
        spool = ctx.enter_context(tc.tile_pool(name="spool", bufs=6))
        opool = ctx.enter_context(tc.tile_pool(name="opool", bufs=1))

        # small inputs first (they gate warmup + everything else)
        whsb = wpool.tile([128, (KCH + 2) * MC], fp16)
        nc.sync.dma_start(whsb, wb)
        whs = whsb[:, 0:KCH * MC].rearrange("p (k n) -> p k n", k=KCH)
        bst2 = whsb[0:1, KCH * MC:(KCH + 2) * MC]  # bias pattern, twice
        ones1 = wpool.tile([1, 128], fp16)
        nc.gpsimd.memset(ones1, 1.0)

        # x: one persistent SBUF tile, streamed in contiguous chunks.
        # layout per chunk: [128, KCH, chs] at flat offset 4*choff[c]
        xt = xpool.tile([128, KCH * BL], fp16)
        # 3 DMA rings in parallel: each ring pays a ~2us completion receipt
        # per transfer (size-independent), so one ring moves only ~1MB/4.5us.
        # c0 leads the scalar ring (nothing ahead of it -> earliest landing);
        # gpsimd's issues all happen before its first tree op (~13us), and
        # each chunk lands before the unit stream reaches it.
        ring = [nc.scalar, nc.sync, nc.scalar, nc.sync, nc.scalar]
        for ci, chs in enumerate(CHUNKS):
            o = KCH * choff[ci]
            n = KCH * chs
            ring[ci].dma_start(xt[:, o:o + n], xh[:, o:o + n])

        def x_tile(ci, s_in_chunk, k):
            # lhsT [128, 128] for k-chunk k, samples s..s+127 of chunk ci
            o = KCH * choff[ci] + k * CHUNKS[ci] + s_in_chunk
            return xt[:, o:o + 128]

        # PE HAM warmup: junk matmuls on W data into a to-be-recycled PSUM
        # bank; keeps the clock at 2.4 GHz by the time real tiles arrive.
        ps = ppool.tile([128, 2 * 512], fp32)  # same tag as the unit tiles
        for i in range(WARMUP_MM):
            nc.tensor.matmul(ps[:, 0:MC], lhsT=whs[:, 0, 0:128],
                             rhs=whs[:, i % KCH, :], start=True, stop=True)

        outbuf = opool.tile([128, NU, NJU, C], fp16)
        JOFF = (0, MC, 512, 512 + MC)   # j-tile offsets inside the 2-bank ps
        stash = {}   # per-unit (t, t3, mx) awaiting the compare stage
        stash2 = {}  # per-unit p2 awaiting the tree tail (one unit later)

        def emit_front(u):
            """Matmuls + PSUM staging + max + broadcast for unit u."""
            ci, s0 = unit_src[u]
            ps = ppool.tile([128, 2 * 512], fp32)
            # one bias matmul per bank covers two j-tiles (N=320)
            for bank in range(2):
                nc.tensor.matmul(ps[:, bank * 512:bank * 512 + 2 * MC],
                                 lhsT=ones1, rhs=bst2,
                                 start=True, stop=False)
            # (k, j) interleave: consecutive matmuls hit different PSUM
            # regions, so one accumulation's drain overlaps the next's fill
            for k in range(KCH):
                for jj in range(NJU):
                    po = ps[:, JOFF[jj]:JOFF[jj] + MC]
                    nc.tensor.matmul(
                        po, lhsT=x_tile(ci, s0 + jj * 128, k),
                        rhs=whs[:, k, :], start=False, stop=(k == KCH - 1))

            # stage the whole unit PSUM -> SBUF fp16 in one strided copy
            t = tpool.tile([128, NJU, M, C], fp16)
            ps_v = (ps.rearrange("p (b x) -> p b x", b=2)[:, :, 0:2 * MC]
                    .rearrange("p b (j n) -> p b j n", j=2))
            nc.scalar.copy(t.rearrange("p (b j) m c -> p b j (m c)", b=2),
                           ps_v)

            t3 = t.rearrange("p j m c -> p (j m) c")
            mx = mpool.tile([128, NJU * M], fp16)
            nc.vector.reduce_max(mx, t3, axis=mybir.AxisListType.X)
            stash[u] = (t, t3, mx)

        def emit_back(u, drain=False):
            """Broadcast + compare + vote-histogram tree for unit u.
            Emitted one unit late so the ACT queue's copy(u+1) is never
            stuck behind bcast(u) (which waits on DVE's max)."""
            t, t3, mx = stash.pop(u)
            mxr = mpool.tile([128, NJU * M, C], fp16)
            nc.scalar.copy(mxr,
                           mx.unsqueeze(2).broadcast_to((128, NJU * M, C)))
            ge = gpool.tile([128, NJU, M, C], bf16)
            nc.vector.tensor_tensor(
                ge.rearrange("p j m c -> p (j m) c"), t3,
                mxr.rearrange("p x c -> p (x c)"),
                mybir.AluOpType.is_ge)
            # histogram tree: GpSimd in steady state (overlaps DVE work on
            # other units); DVE-only in drain mode (contention-free 2x adds
            # beat the cross-engine pipeline when nothing else runs)
            eng = nc.vector if drain else nc.gpsimd
            with nc.allow_low_precision("histogram counts are small ints"):
                p1 = spool.tile([128, NJU, 8, C], bf16)
                eng.tensor_tensor(p1, ge[:, :, 0:8, :], ge[:, :, 8:16, :],
                                  mybir.AluOpType.add)
                p2 = spool.tile([128, NJU, 4, C], bf16)
                eng.tensor_tensor(p2, p1[:, :, 0:4, :], p1[:, :, 4:8, :],
                                  mybir.AluOpType.add)
            stash2[u] = p2

        def emit_tail(u, drain=False):
            """Final 4->1 sum, one unit behind the tree levels so the DVE
            queue never waits on a cross-engine round-trip. In drain mode
            GpSimd is idle, so it takes the adds off the DVE chain."""
            p2 = stash2.pop(u)
            with nc.allow_low_precision("histogram counts are small ints"):
                if drain:
                    p3 = spool.tile([128, NJU, 2, C], bf16)
                    nc.gpsimd.tensor_tensor(p3, p2[:, :, 0:2, :],
                                            p2[:, :, 2:4, :],
                                            mybir.AluOpType.add)
                    nc.gpsimd.tensor_tensor(outbuf[:, u], p3[:, :, 0, :],
                                            p3[:, :, 1, :],
                                            mybir.AluOpType.add)
                else:
                    nc.vector.reduce_sum(
                        outbuf[:, u], p2.rearrange("p j m c -> p j c m"),
                        axis=mybir.AxisListType.X)
            if u % 4 == 3:  # stream the finished group out
                g = u // 4
                nc.sync.dma_start(
                    out[:, g * 4 * NJU * C:(g + 1) * 4 * NJU * C],
                    outbuf[:, g * 4:(g + 1) * 4].rearrange(
                        "p u j c -> p (u j c)"))

        # two-unit software pipeline: unit u's broadcast/compare stage is
        # emitted after unit u+2's front, so every queue always has ready
        # work while a unit's cross-engine round-trips are in flight
        for u in range(NU):
            emit_front(u)
            if u > 2:
                emit_back(u - 3, drain=(u - 3 >= NU - 6))
            if u > 3:
                emit_tail(u - 4, drain=(u - 4 >= NU - 5))
        for u in range(NU - 3, NU):
            emit_back(u, drain=True)
            emit_tail(u - 1, drain=True)
        emit_tail(NU - 1, drain=True)

    nc.compile()
    _NC_CACHE[key] = nc
    return nc


def make_in_maps(x, W, b, ncores=NCORES):
    """Host-side prep: fp16 cast + partition-major relayout + sharding."""
    x16 = np.asarray(x, dtype=np.float32).astype(np.float16)        # [B, D]
    w16 = np.ascontiguousarray(
        np.asarray(W, dtype=np.float32).transpose(1, 0, 2).reshape(D, MC)
    ).astype(np.float16)                                            # [D, 160]
    b16 = np.asarray(b, dtype=np.float32).reshape(MC).astype(np.float16)

    # w dev layout: [p, k, mc] with d = k*128 + p
    w_dev = (w16.reshape(KCH, 128, MC).transpose(1, 0, 2)
             .reshape(128, KCH * MC))
    b_rep = np.broadcast_to(np.concatenate([b16, b16]), (128, 2 * MC))
    wb_dev = np.ascontiguousarray(
        np.concatenate([w_dev, b_rep], axis=1))

    in_maps = []
    for ci in range(ncores):
        xc = x16[ci * BL:(ci + 1) * BL]                             # [BL, D]
        # per chunk: [128, KCH, chs] with dev[p, k, s] = x[b0+s, k*128+p]
        slabs = []
        o = 0
        for chs in CHUNKS:
            slab = xc[o:o + chs].reshape(chs, KCH, 128).transpose(2, 1, 0)
            slabs.append(np.ascontiguousarray(slab).reshape(128, KCH * chs))
            o += chs
        xd = np.ascontiguousarray(np.concatenate(slabs, axis=1))
        in_maps.append({"xh": xd, "wb": wb_dev})
    return in_maps


def kernel(x, W, b):
    global LAST_RESULT
    from concourse import bass_utils

    # NTFF tracing under axon needs the antenv.axon_hooks shim; without it
    # run_bass_kernel_spmd(trace=True) raises. Disable tracing defensively
    # when the hook module is absent (BASS_TRACE may be set in the env).
    want_trace = bool(os.environ.get("BASS_TRACE"))
    try:
        from antenv.axon_hooks import get_axon_ntff_profile_hook  # noqa: F401
    except ImportError:
        want_trace = False
        os.environ["BASS_NEVER_TRACE"] = "1"

    in_maps = make_in_maps(x, W, b)
    nc = build_nc()
    res = bass_utils.run_bass_kernel_spmd(
        nc, in_maps, core_ids=list(range(NCORES)),
        trace=want_trace,
    )
    LAST_RESULT = res
    # device out [128, NU, NJU, C] -> rows b = u*US + j*128 + p
    outs = []
    for r in res.results:
        o = r["out"].reshape(128, NU, NJU, C)
        outs.append(o.transpose(1, 2, 0, 3).reshape(BL, C))
    return np.concatenate(outs, axis=0).astype(np.float32)
